# revision 6
# baseline (speedup 1.0000x reference)
"""Multi-scale patch pooling (gather + segment-mean) for CLIP-AD on 8 trn2 cores.

The reference computes, per batch element b:
    large[b, g, :] = mean over l of tokens[b, large_mask[l, g], :]   (9-elt mean, 169 groups)
    mid[b, g, :]   = mean over l of tokens[b, mid_mask[l, g], :]     (4-elt mean, 196 groups)
    cls[b, 0, :]   = mean over t of tokens[b, t, :]                  (225-elt mean)
    out = concat([large, mid, cls], axis=1)                          # [B, 366, D]

Per batch this is exactly out_b = diag(s) @ W01 @ tokens_b where W01 is a tiny
[366, 225] 0/1 membership-count matrix built host-side from the masks (handles
arbitrary / duplicate indices) and s[g] = 1/group_size. The device kernel runs
the matmul on the tensor engine. For full fp32 accuracy at bf16 matmul speed,
tokens are split host-side into bf16 hi + bf16 lo (x ~= hi + lo to ~2^-18 rel);
W01 entries are small integers, exact in bf16. The per-group 1/L scale is
applied during the PSUM->SBUF evacuation copy.

Sharding: pure data parallel on batch — 64 batches per core; weights replicated.
"""

import numpy as np

B, T, D = 512, 225, 896
GL, LL = 169, 9
GM, LM = 196, 4
G = GL + GM + 1  # 366
N_CORES = 8
BP = B // N_CORES  # 64

_K_TILES = ((0, 128), (128, 97))                # contraction over tokens (225)
_M_TILES = ((0, 128), (128, 128), (256, 110))   # output groups (366)
_N_TILES = ((0, 512), (512, 384))               # feature dim (896)

_CACHE = {}


def _get_nc(bp=BP):
    if bp in _CACHE:
        return _CACHE[bp]
    import concourse.bacc as bacc
    import concourse.mybir as mybir
    import concourse.tile as tile

    f32 = mybir.dt.float32
    bf16 = mybir.dt.bfloat16

    nc = bacc.Bacc("TRN2", target_bir_lowering=False, debug=False)
    # tokens2[b, t, :] = concat(hi[b, t, :], lo[b, t, :]) in bf16
    tokens2 = nc.dram_tensor("tokens2", [bp, T, 2 * D], bf16, kind="ExternalInput").ap()
    w01T = nc.dram_tensor("w01T", [T, G], bf16, kind="ExternalInput").ap()
    scale = nc.dram_tensor("scale", [G, 1], f32, kind="ExternalInput").ap()
    out = nc.dram_tensor("out", [bp, G, D], f32, kind="ExternalOutput").ap()

    with tile.TileContext(nc) as tc:
        with (
            tc.tile_pool(name="w", bufs=1) as wpool,
            tc.tile_pool(name="tok", bufs=4) as tokpool,
            tc.tile_pool(name="ob", bufs=6) as opool,
            tc.tile_pool(name="ps", bufs=3, space="PSUM") as pspool,
        ):
            # Warm-up ops: the first ACT instruction in a kernel picks up an
            # extra activation-table-load wait in lower_act; give it a dummy
            # with no cross-engine deps so real evac copies keep their wait
            # budget. Same idea for the first DVE op.
            warm = wpool.tile([128, 1], f32, tag="warm")
            nc.gpsimd.memset(warm[:], 0.0)
            nc.scalar.activation(
                warm[:], warm[:], mybir.ActivationFunctionType.Copy
            )
            nc.vector.tensor_copy(warm[:], warm[:])

            w_sb = []
            for ki, (k0, ksz) in enumerate(_K_TILES):
                wt = wpool.tile([128, G], bf16, tag=f"w{ki}")
                nc.sync.dma_start(wt[:ksz, :], w01T[k0 : k0 + ksz, :])
                w_sb.append(wt)
            sc_sb = []
            for mi, (m0, msz) in enumerate(_M_TILES):
                st = wpool.tile([128, 1], f32, tag=f"sc{mi}")
                nc.sync.dma_start(st[:msz, :], scale[m0 : m0 + msz, :])
                sc_sb.append(st)

            cp = 0
            for b in range(bp):
                toks = []
                for ki, (k0, ksz) in enumerate(_K_TILES):
                    tk = tokpool.tile([128, 2 * D], bf16, tag=f"tok{ki}")
                    nc.sync.dma_start(tk[:ksz, :], tokens2[b, k0 : k0 + ksz, :])
                    toks.append(tk)
                for mi, (m0, msz) in enumerate(_M_TILES):
                    ps = pspool.tile([128, 1024], f32)
                    for ki, (k0, ksz) in enumerate(_K_TILES):
                        for part in range(2):  # hi, lo
                            for n0, nsz in _N_TILES:
                                nc.tensor.matmul(
                                    ps[:msz, n0 : n0 + nsz],
                                    w_sb[ki][:ksz, m0 : m0 + msz],
                                    toks[ki][:ksz, part * D + n0 : part * D + n0 + nsz],
                                    start=(ki == 0 and part == 0),
                                    stop=(ki == len(_K_TILES) - 1 and part == 1),
                                )
                    o = opool.tile([128, D], f32)
                    # PSUM -> SBUF evacuation with the per-group 1/L scale.
                    # DMA cannot read PSUM.
                    nc.scalar.activation(
                        o[:msz, :],
                        ps[:msz, 0:D],
                        mybir.ActivationFunctionType.Copy,
                        scale=sc_sb[mi][:msz, :],
                    )
                    cp += 1
                    nc.sync.dma_start(out[b, m0 : m0 + msz, :], o[:msz, :])

    nc.compile()
    _CACHE[bp] = nc
    return nc


def _host_prep(tokens_full, large_mask, mid_mask):
    """Split tokens into bf16 hi/lo and build the 0/1 weight + scale tensors."""
    import ml_dtypes

    bf16 = ml_dtypes.bfloat16
    hi = tokens_full.astype(bf16)
    lo = (tokens_full - hi.astype(np.float32)).astype(bf16)
    tokens2 = np.concatenate([hi, lo], axis=-1)  # [B, T, 2D] bf16

    W = np.zeros((G, T), np.float32)
    rows = np.arange(GL)
    for l in range(large_mask.shape[0]):
        np.add.at(W, (rows, large_mask[l]), 1.0)
    rows = GL + np.arange(GM)
    for l in range(mid_mask.shape[0]):
        np.add.at(W, (rows, mid_mask[l]), 1.0)
    W[G - 1, :] = 1.0
    w01T = np.ascontiguousarray(W.T).astype(bf16)  # [T, G], small ints: exact

    s = np.empty((G, 1), np.float32)
    s[:GL] = 1.0 / large_mask.shape[0]
    s[GL : GL + GM] = 1.0 / mid_mask.shape[0]
    s[G - 1] = 1.0 / T
    return tokens2, w01T, s


def _in_maps(tokens2, w01T, s, n_cores=N_CORES):
    bp = tokens2.shape[0] // n_cores
    return [
        {"tokens2": tokens2[c * bp : (c + 1) * bp], "w01T": w01T, "scale": s}
        for c in range(n_cores)
    ]


def kernel(**inputs):
    from concourse import bass_utils

    tokens_full = np.ascontiguousarray(np.asarray(inputs["patch_tokens"], np.float32))
    large = np.asarray(inputs["large_mask"]).astype(np.int64)
    mid = np.asarray(inputs["mid_mask"]).astype(np.int64)
    tokens2, w01T, s = _host_prep(tokens_full, large, mid)

    nc = _get_nc()
    res = bass_utils.run_bass_kernel_spmd(
        nc, _in_maps(tokens2, w01T, s), core_ids=list(range(N_CORES))
    )
    return np.concatenate(
        [res.results[c]["out"] for c in range(N_CORES)], axis=0
    ).astype(np.float32)


# revision 9
# speedup vs baseline: 1.0439x; 1.0439x over previous
"""Multi-scale patch pooling (gather + segment-mean) for CLIP-AD on 8 trn2 cores.

The reference computes, per batch element b:
    large[b, g, :] = mean over l of tokens[b, large_mask[l, g], :]   (9-elt mean, 169 groups)
    mid[b, g, :]   = mean over l of tokens[b, mid_mask[l, g], :]     (4-elt mean, 196 groups)
    cls[b, 0, :]   = mean over t of tokens[b, t, :]                  (225-elt mean)
    out = concat([large, mid, cls], axis=1)                          # [B, 366, D]

Per batch this is exactly out_b = diag(s) @ W01 @ tokens_b where W01 is a tiny
[366, 225] 0/1 membership-count matrix built host-side from the masks (handles
arbitrary / duplicate indices) and s[g] = 1/group_size. The device kernel runs
the matmul on the tensor engine. For full fp32 accuracy at bf16 matmul speed,
tokens are split host-side into bf16 hi + bf16 lo (x ~= hi + lo to ~2^-18 rel);
W01 entries are small integers, exact in bf16. The per-group 1/L scale is
applied during the PSUM->SBUF evacuation copy.

Sharding: pure data parallel on batch — 64 batches per core; weights replicated.
"""

import numpy as np

B, T, D = 512, 225, 896
GL, LL = 169, 9
GM, LM = 196, 4
G = GL + GM + 1  # 366
N_CORES = 8
BP = B // N_CORES  # 64

_K_TILES = ((0, 128), (128, 97))                # contraction over tokens (225)
_M_TILES = ((0, 128), (128, 128), (256, 110))   # output groups (366)
_N_TILES = ((0, 512), (512, 384))               # feature dim (896)

_CACHE = {}


def _get_nc(bp=BP):
    if bp in _CACHE:
        return _CACHE[bp]
    import concourse.bacc as bacc
    import concourse.mybir as mybir
    import concourse.tile as tile

    f32 = mybir.dt.float32
    bf16 = mybir.dt.bfloat16

    nc = bacc.Bacc("TRN2", target_bir_lowering=False, debug=False)
    # tokens2[b, t, :] = concat(hi[b, t, :], lo[b, t, :]) in bf16
    tokens2 = nc.dram_tensor("tokens2", [bp, T, 2 * D], bf16, kind="ExternalInput").ap()
    w01T = nc.dram_tensor("w01T", [T, G], bf16, kind="ExternalInput").ap()
    scale = nc.dram_tensor("scale", [G, 1], f32, kind="ExternalInput").ap()
    out = nc.dram_tensor("out", [bp, G, D], f32, kind="ExternalOutput").ap()

    with tile.TileContext(nc) as tc:
        with (
            tc.tile_pool(name="w", bufs=1) as wpool,
            tc.tile_pool(name="tok", bufs=6) as tokpool,
            tc.tile_pool(name="ob", bufs=9) as opool,
            tc.tile_pool(name="ps", bufs=4, space="PSUM") as pspool,
        ):
            # Warm-up ops: the first ACT instruction in a kernel picks up an
            # extra activation-table-load wait in lower_act; give it a dummy
            # with no cross-engine deps so real evac copies keep their wait
            # budget. Same idea for the first DVE op.
            warm = wpool.tile([128, 1], f32, tag="warm")
            nc.gpsimd.memset(warm[:], 0.0)
            nc.scalar.activation(
                warm[:], warm[:], mybir.ActivationFunctionType.Copy
            )
            nc.vector.tensor_copy(warm[:], warm[:])

            w_sb = []
            for ki, (k0, ksz) in enumerate(_K_TILES):
                wt = wpool.tile([128, G], bf16, tag=f"w{ki}")
                nc.sync.dma_start(wt[:ksz, :], w01T[k0 : k0 + ksz, :])
                w_sb.append(wt)
            sc_sb = []
            for mi, (m0, msz) in enumerate(_M_TILES):
                st = wpool.tile([128, 1], f32, tag=f"sc{mi}")
                nc.sync.dma_start(st[:msz, :], scale[m0 : m0 + msz, :])
                sc_sb.append(st)

            # Spread DMA traffic across the two HWDGE rings (sync=SP ring,
            # scalar=ACT ring) plus the gpsimd SWDGE path so transfers from
            # different streams run in parallel instead of serializing on a
            # single FIFO ring.
            in_eng = [nc.sync, nc.scalar]
            out_eng = [nc.gpsimd, nc.sync, nc.scalar]

            cp = 0
            for b in range(bp):
                toks = []
                for ki, (k0, ksz) in enumerate(_K_TILES):
                    tk = tokpool.tile([128, 2 * D], bf16, tag=f"tok{ki}")
                    in_eng[ki].dma_start(tk[:ksz, :], tokens2[b, k0 : k0 + ksz, :])
                    toks.append(tk)
                for mi, (m0, msz) in enumerate(_M_TILES):
                    ps = pspool.tile([128, 1024], f32)
                    for ki, (k0, ksz) in enumerate(_K_TILES):
                        for part in range(2):  # hi, lo
                            for n0, nsz in _N_TILES:
                                nc.tensor.matmul(
                                    ps[:msz, n0 : n0 + nsz],
                                    w_sb[ki][:ksz, m0 : m0 + msz],
                                    toks[ki][:ksz, part * D + n0 : part * D + n0 + nsz],
                                    start=(ki == 0 and part == 0),
                                    stop=(ki == len(_K_TILES) - 1 and part == 1),
                                )
                    o = opool.tile([128, D], f32)
                    # PSUM -> SBUF evacuation with the per-group 1/L scale.
                    # DMA cannot read PSUM; alternate DVE / ACT so neither
                    # engine becomes the bottleneck.
                    if cp % 3 == 1:
                        nc.scalar.activation(
                            o[:msz, :],
                            ps[:msz, 0:D],
                            mybir.ActivationFunctionType.Copy,
                            scale=sc_sb[mi][:msz, :],
                        )
                    else:
                        nc.vector.tensor_scalar_mul(
                            o[:msz, :], ps[:msz, 0:D], sc_sb[mi][:msz, :]
                        )
                    cp += 1
                    out_eng[mi].dma_start(out[b, m0 : m0 + msz, :], o[:msz, :])

    nc.compile()
    _CACHE[bp] = nc
    return nc


def _host_prep(tokens_full, large_mask, mid_mask):
    """Split tokens into bf16 hi/lo and build the 0/1 weight + scale tensors."""
    import ml_dtypes

    bf16 = ml_dtypes.bfloat16
    hi = tokens_full.astype(bf16)
    lo = (tokens_full - hi.astype(np.float32)).astype(bf16)
    tokens2 = np.concatenate([hi, lo], axis=-1)  # [B, T, 2D] bf16

    W = np.zeros((G, T), np.float32)
    rows = np.arange(GL)
    for l in range(large_mask.shape[0]):
        np.add.at(W, (rows, large_mask[l]), 1.0)
    rows = GL + np.arange(GM)
    for l in range(mid_mask.shape[0]):
        np.add.at(W, (rows, mid_mask[l]), 1.0)
    W[G - 1, :] = 1.0
    w01T = np.ascontiguousarray(W.T).astype(bf16)  # [T, G], small ints: exact

    s = np.empty((G, 1), np.float32)
    s[:GL] = 1.0 / large_mask.shape[0]
    s[GL : GL + GM] = 1.0 / mid_mask.shape[0]
    s[G - 1] = 1.0 / T
    return tokens2, w01T, s


def _in_maps(tokens2, w01T, s, n_cores=N_CORES):
    bp = tokens2.shape[0] // n_cores
    return [
        {"tokens2": tokens2[c * bp : (c + 1) * bp], "w01T": w01T, "scale": s}
        for c in range(n_cores)
    ]


def kernel(**inputs):
    from concourse import bass_utils

    tokens_full = np.ascontiguousarray(np.asarray(inputs["patch_tokens"], np.float32))
    large = np.asarray(inputs["large_mask"]).astype(np.int64)
    mid = np.asarray(inputs["mid_mask"]).astype(np.int64)
    tokens2, w01T, s = _host_prep(tokens_full, large, mid)

    nc = _get_nc()
    res = bass_utils.run_bass_kernel_spmd(
        nc, _in_maps(tokens2, w01T, s), core_ids=list(range(N_CORES))
    )
    return np.concatenate(
        [res.results[c]["out"] for c in range(N_CORES)], axis=0
    ).astype(np.float32)
